# revision 3
# baseline (speedup 1.0000x reference)
"""Trainium2 Bass kernel for nn_AttentionBlock (B=4, L=2048, C=512, H=8, Dk=64).

Sharding (8 cores): data-parallel over B (4) x tensor-parallel over heads (2
groups of 4). Core c handles batch c//2, head group c%2. Each core computes
  y_c = attention(x_b)[:, local_heads] @ W_out[local_rows]        [2048, 512]
and the host combines: out[b] = y[2b] + y[2b+1] + b_out + x[b].

Device kernel (per core), all matmuls on PE with fp32 PSUM accumulation:
  - qT/kT per head in [Dk, L] layout straight out of the projection
    (lhsT=W_in chunk, rhs=xT chunk) -- no transposes anywhere.
  - v in natural [L, Dk] layout, augmented with a ones column so the
    O^T = V^T P^T matmul also produces the softmax denominators for free.
  - scores S^T [keys, queries] tile-by-tile; causal structure skips
    upper-triangle tiles; diagonal straddle tiles get a precomputed
    0/1 mask multiply after exp.
  - softmax normalization: reciprocal of denom row + K=1 ones-matmul to
    broadcast it across partitions + one DVE multiply.
"""

import sys

sys.path.insert(0, "/opt/trn_rl_repo")

import numpy as np

import concourse.bacc as bacc
import concourse.bass as bass
import concourse.mybir as mybir
import concourse.tile as tile
from concourse.bass_utils import run_bass_kernel_spmd

# ---------------------------------------------------------------- constants
B, L, C = 4, 2048, 512
H, DK = 8, 64
HPC = 4  # heads per core
SCALE = DK**-0.5
N_CORES = 8
KC = C // 128  # 4 contraction chunks
LT = L // 128  # 16 row tiles
QB = L // 512  # 4 query blocks of 512

F32 = mybir.dt.float32
BF16 = mybir.dt.bfloat16

# matmul operand dtype: "bf16" (fast) or "fp32" (exact)
MM_MODE = "bf16"

# test hooks (grading path leaves these alone)
TRACE = False
LAST_RESULT = None

_CACHE = {}


def _np_mm_dtype():
    if MM_MODE == "bf16":
        import ml_dtypes

        return ml_dtypes.bfloat16
    return np.float32


def _build(mm_mode):
    mm = BF16 if mm_mode == "bf16" else F32
    nc = bacc.Bacc(None)

    xT = nc.declare_dram_parameter("xT", [C, L], mm, isOutput=False)
    w_in = nc.declare_dram_parameter("w_in", [C, HPC, 192], mm, isOutput=False)
    qkb = nc.declare_dram_parameter("qkb", [64, 8], F32, isOutput=False)
    vb = nc.declare_dram_parameter("vb", [HPC, DK + 1], F32, isOutput=False)
    w_out = nc.declare_dram_parameter("w_out", [HPC, DK, C], mm, isOutput=False)
    masks = nc.declare_dram_parameter("masks", [128, 4, 512], mm, isOutput=False)
    y = nc.declare_dram_parameter("y", [L, C], F32, isOutput=True)

    with tile.TileContext(nc) as tc:
        with (
            tc.tile_pool(name="persist", bufs=1) as per,
            tc.tile_pool(name="work", bufs=2) as work,
            tc.tile_pool(name="psum", bufs=1, space="PSUM") as psum,
        ):
            # ---------------- loads
            xT_sb = [per.tile([128, L], mm, tag=f"xT{i}", name=f"xT{i}") for i in range(KC)]
            w_in_sb = [per.tile([128, HPC, 192], mm, tag=f"wi{i}", name=f"wi{i}") for i in range(KC)]
            w_out_sb = [per.tile([DK, C], mm, tag=f"wo{h}", name=f"wo{h}") for h in range(HPC)]
            masks_sb = per.tile([128, 4, 512], mm, tag="masks")
            qkb_sb = per.tile([64, 8], F32, tag="qkb")
            vb_sb = per.tile([128, HPC, DK + 1], F32, tag="vb")
            ones_rb = per.tile([1, DK], F32, tag="ones_rb")

            xT_t = xT.rearrange("(c p) l -> c p l", p=128)
            w_in_t = w_in.rearrange("(c p) h d -> c p h d", p=128)
            for i in range(KC):
                nc.sync.dma_start(out=xT_sb[i], in_=xT_t[i])
                nc.sync.dma_start(out=w_in_sb[i], in_=w_in_t[i])
            for h in range(HPC):
                nc.sync.dma_start(out=w_out_sb[h], in_=w_out[h])
            nc.sync.dma_start(out=masks_sb, in_=masks[:, :, :])
            nc.sync.dma_start(out=qkb_sb, in_=qkb[:, :])
            vb_ap = vb[:, :]
            vb_bcast = bass.AP(
                tensor=vb_ap.tensor, offset=vb_ap.offset, ap=[[0, 128], *vb_ap.ap]
            )
            nc.sync.dma_start(out=vb_sb, in_=vb_bcast)
            nc.vector.memset(ones_rb, 1.0)

            # ---------------- q/k projection: qT/kT [Dk, L] per head
            qT_sb = [per.tile([DK, L], mm, tag=f"qT{h}", name=f"qT{h}") for h in range(HPC)]
            kT_sb = [per.tile([DK, L], mm, tag=f"kT{h}", name=f"kT{h}") for h in range(HPC)]
            for m in range(2 * HPC):
                h, half = divmod(m, 2)
                dst_t = qT_sb[h] if half == 0 else kT_sb[h]
                for lc in range(L // 512):
                    ps = psum.tile([DK, 512], F32, tag="mm", bufs=2)
                    for kc in range(KC):
                        nc.tensor.matmul(
                            ps,
                            lhsT=w_in_sb[kc][:, h, 64 * half : 64 * half + 64],
                            rhs=xT_sb[kc][:, lc * 512 : (lc + 1) * 512],
                            start=(kc == 0),
                            stop=(kc == KC - 1),
                        )
                    nc.vector.tensor_scalar_add(
                        dst_t[:, lc * 512 : (lc + 1) * 512], ps, qkb_sb[:, m : m + 1]
                    )

            # ---------------- v projection: natural [L, Dk] + ones column
            v_sb = [per.tile([128, HPC, DK + 1], mm, tag=f"v{lt}", name=f"v{lt}") for lt in range(LT)]
            for lt in range(LT):
                ps = psum.tile([128, HPC, DK], F32, tag="mm", bufs=2)
                for kc in range(KC):
                    nc.tensor.matmul(
                        ps,
                        lhsT=xT_sb[kc][:, lt * 128 : (lt + 1) * 128],
                        rhs=w_in_sb[kc][:, :, 128:192],
                        start=(kc == 0),
                        stop=(kc == KC - 1),
                    )
                nc.vector.tensor_add(
                    v_sb[lt][:, :, 0:DK], ps, vb_sb[:, :, 0:DK]
                )
                nc.vector.memset(v_sb[lt][:, :, DK : DK + 1], 1.0)

            # ---------------- attention (S^T layout, causal)
            ot_sb = [per.tile([DK, L], mm, tag=f"ot{h}", name=f"ot{h}") for h in range(HPC)]
            for h in range(HPC):
                for qb in range(QB):
                    ot = psum.tile([DK + 1, 512], F32, tag="ot", bufs=2)
                    nkj = 4 * qb + 4
                    for kj in range(nkj):
                        st = psum.tile([128, 512], F32, tag="st", bufs=3)
                        nc.tensor.matmul(
                            st,
                            lhsT=kT_sb[h][:, kj * 128 : (kj + 1) * 128],
                            rhs=qT_sb[h][:, qb * 512 : (qb + 1) * 512],
                            start=True,
                            stop=True,
                        )
                        se = work.tile([128, 512], mm, tag="se", bufs=3)
                        nc.scalar.activation(
                            out=se,
                            in_=st,
                            func=mybir.ActivationFunctionType.Exp,
                            scale=float(SCALE),
                        )
                        r = kj - 4 * qb
                        if r >= 0:
                            nc.vector.tensor_mul(se, se, masks_sb[:, r, :])
                        nc.tensor.matmul(
                            ot,
                            lhsT=v_sb[kj][:, h, :],
                            rhs=se,
                            start=(kj == 0),
                            stop=(kj == nkj - 1),
                        )
                    # normalize: ot[:DK] / ot[DK]
                    recip = work.tile([1, 512], F32, tag="recip", bufs=2)
                    nc.vector.reciprocal(recip, ot[DK : DK + 1, :])
                    rb = psum.tile([DK, 512], F32, tag="rb", bufs=1)
                    nc.tensor.matmul(rb, lhsT=ones_rb, rhs=recip, start=True, stop=True)
                    rbs = work.tile([DK, 512], F32, tag="rbs", bufs=2)
                    nc.vector.tensor_copy(rbs, rb)
                    nc.vector.tensor_mul(
                        ot_sb[h][:, qb * 512 : (qb + 1) * 512], ot[0:DK, :], rbs
                    )

            # ---------------- output projection + store
            for lt in range(LT):
                yp = psum.tile([128, C], F32, tag="mm", bufs=2)
                for h in range(HPC):
                    nc.tensor.matmul(
                        yp,
                        lhsT=ot_sb[h][:, lt * 128 : (lt + 1) * 128],
                        rhs=w_out_sb[h],
                        start=(h == 0),
                        stop=(h == HPC - 1),
                    )
                ysb = work.tile([128, C], F32, tag="ysb", bufs=3)
                nc.vector.tensor_copy(ysb, yp)
                nc.sync.dma_start(out=y[lt * 128 : (lt + 1) * 128, :], in_=ysb)

    nc.finalize()
    return nc


def _get_nc():
    if MM_MODE not in _CACHE:
        _CACHE[MM_MODE] = _build(MM_MODE)
    return _CACHE[MM_MODE]


def _make_masks():
    tri = np.triu(np.ones((128, 128), np.float32))  # [j, i] = 1 iff i >= j
    m = np.ones((128, 4, 512), np.float32)
    for r in range(4):
        for s in range(4):
            if s < r:
                m[:, r, s * 128 : (s + 1) * 128] = 0.0
            elif s == r:
                m[:, r, s * 128 : (s + 1) * 128] = tri
    return m


def kernel(x, W_in, b_in, W_out, b_out):
    x = np.asarray(x, np.float32)
    W_in = np.asarray(W_in, np.float32)
    b_in = np.asarray(b_in, np.float32)
    W_out = np.asarray(W_out, np.float32)
    b_out = np.asarray(b_out, np.float32)

    mmd = _np_mm_dtype()
    masks = _make_masks().astype(mmd)

    in_maps = []
    for c in range(N_CORES):
        b, j = divmod(c, 2)
        w_in_loc = W_in[:, j * 768 : (j + 1) * 768]  # [C, 768]
        b_in_loc = b_in[j * 768 : (j + 1) * 768]  # [768]
        xT = np.ascontiguousarray(x[b].T).astype(mmd)  # [C, L]
        w_in_3d = np.ascontiguousarray(w_in_loc.reshape(C, HPC, 192)).astype(mmd)
        qkb = np.empty((64, 8), np.float32)
        for m in range(8):
            h, half = divmod(m, 2)
            o = 192 * h + 64 * half
            qkb[:, m] = b_in_loc[o : o + 64]
        vb = np.zeros((HPC, DK + 1), np.float32)
        for h in range(HPC):
            vb[h, :DK] = b_in_loc[192 * h + 128 : 192 * h + 192]
        w_out_loc = np.empty((HPC, DK, C), np.float32)
        for h in range(HPC):
            hh = j * HPC + h
            w_out_loc[h] = W_out[hh * DK : (hh + 1) * DK, :]
        in_maps.append(
            dict(
                xT=xT,
                w_in=w_in_3d,
                qkb=qkb,
                vb=vb,
                w_out=w_out_loc.astype(mmd),
                masks=masks,
            )
        )

    nc = _get_nc()
    res = run_bass_kernel_spmd(
        nc, in_maps, core_ids=list(range(N_CORES)), trace=TRACE
    )
    global LAST_RESULT
    LAST_RESULT = res

    out = np.empty((B, L, C), np.float32)
    for b in range(B):
        out[b] = (
            res.results[2 * b]["y"]
            + res.results[2 * b + 1]["y"]
            + b_out[None, :]
            + x[b]
        )
    return out
